# revision 10
# baseline (speedup 1.0000x reference)
"""Trainium2 Bass kernel for nn_Attention_54254026883778.

Single-head attention with an additive post-softmax intensity term:
    q/k/v = X @ W{q,k,v}.T + b;  scores = q k^T / sqrt(D)
    attn  = softmax(scores) + intensity;  out = (attn @ v) @ Wo.T + bo

Sharding: 8 cores = 4 batches x 2 sequence halves. Each core computes
K^T and V for its whole batch (duplicated across the pair) and Q/attention
for its own 1024 query rows. No collectives. The host rolls the t-axis per
core so the core's own query rows are always at t-positions 0..SH; K/V and
intensity follow the same rolled order, which leaves attn @ V invariant.

Device dataflow (host pre-transposes X and W so the contraction dim lands
on SBUF partitions):
    Q^T/K^T  [dout | s]  = WxT-chunk.T @ XT        (lhsT=WxT, rhs=XT)
    V        [t | dv]    = XT-chunk.T @ WvT        (lhsT=XT,  rhs=WvT)
    scores   [s | t]     = QT-chunk.T @ KT         -> exp (no max-subtract:
        |scores| < ~3) with fused row-accumulate -> 1/den on DVE ->
        ACT copy*recip -> + intensity (DVE) -> PE-transpose -> attn^T [t|s]
    out^T    [dv | s]    = V-chunk.T @ attn^T
    final^T  [do | s]    = WoT-chunk.T @ out^T     -> DRAM, host transposes
Biases enter as rank-1 fp32r matmuls accumulated into PSUM:
    q/k/o:  b_row (x) ones_row;   v: bv_row (x) rowsum(attn)
(the attention rows sum to 1 + rowsum(intensity), so V's bias scales with
the attn row-sum, computed on-device via reduce + PE vector transpose).
"""

import numpy as np
import ml_dtypes

P = 128
D = 1024
S = 2048          # keys per batch (full sequence)
SH = 1024         # query rows owned by each core
DC = D // P       # 8  contraction chunks over model dim
TC = S // P       # 16 t (key) chunks
NT = 512          # matmul moving free dim / psum bank
SJ = SH // NT     # 2  s-tiles of own rows
TJ = S // NT      # 4  t-tiles
SCALE = 1.0 / 32.0  # 1/sqrt(D)

_CACHE = {}


def _build_module():
    import concourse.bass as bass
    import concourse.tile as tile
    import concourse.mybir as mybir
    from concourse import bacc
    from concourse.masks import make_identity

    f32 = mybir.dt.float32
    f32r = mybir.dt.float32r
    bf16 = mybir.dt.bfloat16
    Exp = mybir.ActivationFunctionType.Exp
    add = mybir.AluOpType.add

    nc = bacc.Bacc("TRN2", target_bir_lowering=False, debug=False)

    XT_d = nc.dram_tensor("XT", [D, S], bf16, kind="ExternalInput")
    WQ_d = nc.dram_tensor("WQT", [D, D], bf16, kind="ExternalInput")
    WK_d = nc.dram_tensor("WKT", [D, D], bf16, kind="ExternalInput")
    WV_d = nc.dram_tensor("WVT", [D, D], bf16, kind="ExternalInput")
    WO_d = nc.dram_tensor("WOT", [D, D], bf16, kind="ExternalInput")
    BALL_d = nc.dram_tensor("BALL", [1, 4 * D + NT], f32, kind="ExternalInput")
    INT_d = nc.dram_tensor("INT", [SH, S], f32, kind="ExternalInput")
    OUT_d = nc.dram_tensor("OUTT", [D, SH], f32, kind="ExternalOutput")

    xt_v = XT_d[:].rearrange("(c p) s -> p c s", p=P)
    wq_v = WQ_d[:].rearrange("(c p) o -> p c o", p=P)
    wk_v = WK_d[:].rearrange("(c p) o -> p c o", p=P)
    wv_v = WV_d[:].rearrange("(c p) o -> p c o", p=P)
    wo_v = WO_d[:].rearrange("(c p) o -> p c o", p=P)
    int_v = INT_d[:].rearrange("(c p) t -> p c t", p=P)
    out_v = OUT_d[:].rearrange("(c p) s -> p c s", p=P)

    with tile.TileContext(nc) as tc:
        with (
            tc.tile_pool(name="persist", bufs=1) as persist,
            tc.tile_pool(name="mm_ps", bufs=4, space="PSUM") as mm_ps,
            tc.tile_pool(name="tr_ps", bufs=3, space="PSUM") as tr_ps,
            tc.tile_pool(name="rs_ps", bufs=1, space="PSUM") as rs_ps,
        ):
            # ---- persistent tiles -------------------------------------
            KT_sb = persist.tile([P, DC, S], bf16)          # K^T  [d | t]
            V_sb = persist.tile([P, TC, D], bf16)           # V natural [t | dv]
            QT_sb = persist.tile([P, DC, SH], bf16, tag="qt_ot")   # Q^T [d | s]
            ident = persist.tile([P, P], bf16)
            make_identity(nc, ident)
            identf = persist.tile([P, P], f32)
            make_identity(nc, identf)
            # fp32r matmul operands must come from an instruction that
            # rounds to fp32r -> stage the DMA'd bias pack (bq|bk|bv|bo|ones)
            # through a DVE copy.
            BALL_ld = persist.tile([1, 4 * D + NT], f32)
            nc.sync.dma_start(BALL_ld[:], BALL_d[:])
            BALL_r = persist.tile([1, 4 * D + NT], f32r)
            nc.vector.tensor_copy(out=BALL_r[:], in_=BALL_ld[:])
            ONES = BALL_r[0:1, 4 * D:4 * D + NT]
            BQ_sb = BALL_r[0:1, 0 * D:1 * D]
            BK_sb = BALL_r[0:1, 1 * D:2 * D]
            BV_sb = BALL_r[0:1, 2 * D:3 * D]
            BO_sb = BALL_r[0:1, 3 * D:4 * D]
            RS_sb = persist.tile([1, SH], f32r)             # rowsum(attn)

            # ---- phase A: QKV ----------------------------------------
            # XT shares its slot with attn^T (written only after XT's last
            # read); Wq/Wk/Wv double-buffer in a phase-scoped pool.
            XT_sb = persist.tile([P, DC, S], bf16, tag="xt_at")
            nc.sync.dma_start(XT_sb[:], xt_v)
            with tc.tile_pool(name="wpool", bufs=2) as wpool:
                WQ_sb = wpool.tile([P, DC, D], bf16, tag="w")
                nc.sync.dma_start(WQ_sb[:], wq_v)
                WK_sb = wpool.tile([P, DC, D], bf16, tag="w")
                nc.sync.dma_start(WK_sb[:], wk_v)

                # Q^T [dout, s-own]
                for c in range(DC):
                    for j in range(SJ):
                        ps = mm_ps.tile([P, NT], f32, tag="mm")
                        for dc in range(DC):
                            nc.tensor.matmul(
                                ps[:],
                                WQ_sb[:, dc, c * P:(c + 1) * P],
                                XT_sb[:, dc, j * NT:(j + 1) * NT],
                                start=(dc == 0), stop=False,
                            )
                        nc.tensor.matmul(
                            ps[:], BQ_sb[0:1, c * P:(c + 1) * P],
                            ONES[:], start=False, stop=True)
                        nc.vector.tensor_copy(
                            out=QT_sb[:, c, j * NT:(j + 1) * NT], in_=ps[:])
                # K^T [dout, t-full]
                for c in range(DC):
                    for j in range(TJ):
                        ps = mm_ps.tile([P, NT], f32, tag="mm")
                        for dc in range(DC):
                            nc.tensor.matmul(
                                ps[:],
                                WK_sb[:, dc, c * P:(c + 1) * P],
                                XT_sb[:, dc, j * NT:(j + 1) * NT],
                                start=(dc == 0), stop=False,
                            )
                        nc.tensor.matmul(
                            ps[:], BK_sb[0:1, c * P:(c + 1) * P],
                            ONES[:], start=False, stop=True)
                        nc.vector.tensor_copy(
                            out=KT_sb[:, c, j * NT:(j + 1) * NT], in_=ps[:])
                WV_sb = wpool.tile([P, DC, D], bf16, tag="w")
                nc.sync.dma_start(WV_sb[:], wv_v)
                # V natural [t, dv] (no bias; rank-1 correction at PV)
                for t in range(TC):
                    for j in range(D // NT):
                        ps = mm_ps.tile([P, NT], f32, tag="mm")
                        for dc in range(DC):
                            nc.tensor.matmul(
                                ps[:],
                                XT_sb[:, dc, t * P:(t + 1) * P],
                                WV_sb[:, dc, j * NT:(j + 1) * NT],
                                start=(dc == 0),
                                stop=(dc == DC - 1),
                            )
                        nc.vector.tensor_copy(
                            out=V_sb[:, t, j * NT:(j + 1) * NT], in_=ps[:])

            # ---- phase B/C: scores -> softmax -> +intensity -> transpose
            AT_sb = persist.tile([P, TC, SH], bf16, tag="xt_at")  # attn^T [t|s]
            with (
                tc.tile_pool(name="attn_pool", bufs=2) as attn_pool,
                tc.tile_pool(name="int_pool", bufs=3) as int_pool,
                tc.tile_pool(name="stat_pool", bufs=2) as stat_pool,
            ):
                for si in range(DC):  # 8 chunks of 128 own query rows
                    A_sb = attn_pool.tile([P, S], bf16, tag="attn")
                    acc4 = stat_pool.tile([P, TJ], f32, tag="acc")
                    INT_sb = int_pool.tile([P, S], f32, tag="int")
                    nc.sync.dma_start(INT_sb[:], int_v[:, si, :])
                    for tj in range(TJ):
                        ps = mm_ps.tile([P, NT], f32, tag="mm")
                        for dc in range(DC):
                            nc.tensor.matmul(
                                ps[:],
                                QT_sb[:, dc, si * P:(si + 1) * P],
                                KT_sb[:, dc, tj * NT:(tj + 1) * NT],
                                start=(dc == 0),
                                stop=(dc == DC - 1),
                            )
                        nc.scalar.activation(
                            A_sb[:, tj * NT:(tj + 1) * NT], ps[:], Exp,
                            scale=SCALE, accum_out=acc4[:, tj:tj + 1],
                        )
                    den = stat_pool.tile([P, 1], f32, tag="den")
                    recip = stat_pool.tile([P, 1], f32, tag="recip")
                    nc.vector.reduce_sum(den[:], acc4[:], axis=mybir.AxisListType.X)
                    nc.vector.reciprocal(recip[:], den[:])
                    nc.scalar.mul(A_sb[:], A_sb[:], recip[:])
                    nc.vector.tensor_tensor(A_sb[:], A_sb[:], INT_sb[:], add)
                    # rowsum(attn) for the V-bias rank-1 term
                    rs = stat_pool.tile([P, 1], f32, tag="rs")
                    nc.vector.reduce_sum(rs[:], A_sb[:], axis=mybir.AxisListType.X)
                    pr = rs_ps.tile([1, P], f32, tag="rsps")
                    nc.tensor.transpose(pr[:], rs[:], identf[:])
                    nc.scalar.copy(RS_sb[0:1, si * P:(si + 1) * P], pr[:])
                    # transpose attn tiles -> attn^T
                    for t in range(TC):
                        pt = tr_ps.tile([P, P], bf16, tag="tr")
                        nc.tensor.transpose(
                            pt[:], A_sb[:, t * P:(t + 1) * P], ident[:])
                        nc.scalar.copy(AT_sb[:, t, si * P:(si + 1) * P], pt[:])

            # ---- phase D: PV -> out^T [dv, s] -------------------------
            OT_sb = persist.tile([P, DC, SH], bf16, tag="qt_ot")
            for dvi in range(DC):
                for sj in range(SJ):
                    ps = mm_ps.tile([P, NT], f32, tag="mm")
                    for t in range(TC):
                        nc.tensor.matmul(
                            ps[:],
                            V_sb[:, t, dvi * P:(dvi + 1) * P],
                            AT_sb[:, t, sj * NT:(sj + 1) * NT],
                            start=(t == 0),
                            stop=False,
                        )
                    # bias: bv (x) rowsum(attn)
                    nc.tensor.matmul(
                        ps[:], BV_sb[0:1, dvi * P:(dvi + 1) * P],
                        RS_sb[0:1, sj * NT:(sj + 1) * NT],
                        start=False, stop=True)
                    nc.vector.tensor_copy(
                        out=OT_sb[:, dvi, sj * NT:(sj + 1) * NT], in_=ps[:])

            # ---- phase E: output projection -> final^T [do, s] --------
            with (
                tc.tile_pool(name="wo_pool", bufs=1) as wo_pool,
                tc.tile_pool(name="fin_pool", bufs=3) as fin_pool,
            ):
                WO_sb = wo_pool.tile([P, DC, D], bf16)
                nc.sync.dma_start(WO_sb[:], wo_v)
                for doi in range(DC):
                    for sj in range(SJ):
                        ps = mm_ps.tile([P, NT], f32, tag="mm")
                        for dvc in range(DC):
                            nc.tensor.matmul(
                                ps[:],
                                WO_sb[:, dvc, doi * P:(doi + 1) * P],
                                OT_sb[:, dvc, sj * NT:(sj + 1) * NT],
                                start=(dvc == 0), stop=False,
                            )
                        nc.tensor.matmul(
                            ps[:], BO_sb[0:1, doi * P:(doi + 1) * P],
                            ONES[:], start=False, stop=True)
                        F_sb = fin_pool.tile([P, NT], f32, tag="fin")
                        nc.vector.tensor_copy(out=F_sb[:], in_=ps[:])
                        nc.sync.dma_start(out_v[:, doi, sj * NT:(sj + 1) * NT], F_sb[:])

    nc.compile()
    return nc


def _get_module():
    if "nc" not in _CACHE:
        _CACHE["nc"] = _build_module()
    return _CACHE["nc"]


def _make_in_maps(inputs):
    X = np.asarray(inputs["X"], dtype=np.float32)
    intensity = np.asarray(inputs["intensity"], dtype=np.float32)
    bf = ml_dtypes.bfloat16
    WqT = np.ascontiguousarray(np.asarray(inputs["Wq"], np.float32).T).astype(bf)
    WkT = np.ascontiguousarray(np.asarray(inputs["Wk"], np.float32).T).astype(bf)
    WvT = np.ascontiguousarray(np.asarray(inputs["Wv"], np.float32).T).astype(bf)
    WoT = np.ascontiguousarray(np.asarray(inputs["Wo"], np.float32).T).astype(bf)
    BALL = np.concatenate(
        [np.asarray(inputs[k], np.float32).reshape(D) for k in
         ("bq", "bk", "bv", "bo")] + [np.ones(NT, np.float32)]
    ).reshape(1, 4 * D + NT)

    in_maps = []
    for c in range(8):
        b, h = c // 2, c % 2
        XT = np.ascontiguousarray(np.roll(X[b].T, -h * SH, axis=1)).astype(bf)
        INT = np.ascontiguousarray(
            np.roll(intensity[b, h * SH:(h + 1) * SH, :], -h * SH, axis=1))
        in_maps.append({
            "XT": XT, "WQT": WqT, "WKT": WkT, "WVT": WvT, "WOT": WoT,
            "BALL": BALL, "INT": INT,
        })
    return in_maps


def _gather(results):
    out = np.empty((4, S, D), dtype=np.float32)
    for c in range(8):
        b, h = c // 2, c % 2
        out[b, h * SH:(h + 1) * SH, :] = results[c]["OUTT"].T
    return out


def kernel(**inputs):
    from concourse import bass_utils

    in_maps = _make_in_maps(inputs)
    nc = _get_module()
    res = bass_utils.run_bass_kernel_spmd(nc, in_maps, core_ids=list(range(8)))
    return _gather(res.results)


# revision 11
# speedup vs baseline: 1.0561x; 1.0561x over previous
"""Trainium2 Bass kernel for nn_Attention_54254026883778.

Single-head attention with an additive post-softmax intensity term:
    q/k/v = X @ W{q,k,v}.T + b;  scores = q k^T / sqrt(D)
    attn  = softmax(scores) + intensity;  out = (attn @ v) @ Wo.T + bo

Sharding: 8 cores = 4 batches x 2 sequence halves. Each core computes
K^T and V for its whole batch (duplicated across the pair) and Q/attention
for its own 1024 query rows. No collectives. The host rolls the t-axis per
core so the core's own query rows are always at t-positions 0..SH; K/V and
intensity follow the same rolled order, which leaves attn @ V invariant.

Device dataflow (host pre-transposes X, W and intensity so contraction /
partition dims land where the engines want them):
    Q^T/K^T  [dout | s]  = WxT-chunk.T @ XT        (lhsT=WxT, rhs=XT)
    V        [t | dv]    = XT-chunk.T @ WvT        (lhsT=XT,  rhs=WvT)
    scores   [s | t]     = QT-chunk.T @ KT  -> exp on ACT (no max-subtract:
        |scores| < ~3) with fused row-accumulate -> 1/den on DVE ->
        diag(recip) = ident * recip (DVE) ->
        attn^T tile = E-slice.T @ diag(recip)      (one PE matmul both
        transposes and normalizes) -> DVE copy adds intensity^T (bf16,
        host-transposed) while draining PSUM -> attn^T [t | s]
    out^T    [dv | s]    = V-chunk.T @ attn^T
    final^T  [do | s]    = WoT-chunk.T @ out^T     -> DRAM, host transposes
Biases enter as rank-1 fp32r matmuls accumulated into PSUM:
    q/k/o:  b_row (x) ones_row;   v: bv_row (x) attn-rowsums, where the
    rowsums are 1 + rowsum(intensity) (softmax rows sum to 1), shipped
    from the host inside the bias pack.
"""

import numpy as np
import ml_dtypes

P = 128
D = 1024
S = 2048          # keys per batch (full sequence)
SH = 1024         # query rows owned by each core
DC = D // P       # 8  contraction chunks over model dim
TC = S // P       # 16 t (key) chunks
NT = 512          # matmul moving free dim / psum bank
SJ = SH // NT     # 2  s-tiles of own rows
TJ = S // NT      # 4  t-tiles
SCALE = 1.0 / 32.0  # 1/sqrt(D)
BSZ = 4 * D + NT + SH  # bias pack: bq|bk|bv|bo|ones|rowsums

_CACHE = {}


def _build_module():
    import concourse.bass as bass
    import concourse.tile as tile
    import concourse.mybir as mybir
    from concourse import bacc
    from concourse.masks import make_identity

    f32 = mybir.dt.float32
    f32r = mybir.dt.float32r
    bf16 = mybir.dt.bfloat16
    Exp = mybir.ActivationFunctionType.Exp
    add = mybir.AluOpType.add

    nc = bacc.Bacc("TRN2", target_bir_lowering=False, debug=False)

    XT_d = nc.dram_tensor("XT", [D, S], bf16, kind="ExternalInput")
    WQ_d = nc.dram_tensor("WQT", [D, D], bf16, kind="ExternalInput")
    WK_d = nc.dram_tensor("WKT", [D, D], bf16, kind="ExternalInput")
    WV_d = nc.dram_tensor("WVT", [D, D], bf16, kind="ExternalInput")
    WO_d = nc.dram_tensor("WOT", [D, D], bf16, kind="ExternalInput")
    BALL_d = nc.dram_tensor("BALL", [1, BSZ], f32, kind="ExternalInput")
    IT_d = nc.dram_tensor("IT", [S, SH], bf16, kind="ExternalInput")
    OUT_d = nc.dram_tensor("OUTT", [D, SH], f32, kind="ExternalOutput")

    xt_v = XT_d[:].rearrange("(c p) s -> p c s", p=P)
    wq_v = WQ_d[:].rearrange("(c p) o -> p c o", p=P)
    wk_v = WK_d[:].rearrange("(c p) o -> p c o", p=P)
    wv_v = WV_d[:].rearrange("(c p) o -> p c o", p=P)
    wo_v = WO_d[:].rearrange("(c p) o -> p c o", p=P)
    it_v = IT_d[:].rearrange("(c p) s -> p c s", p=P)   # [t-part, tc, s]
    out_v = OUT_d[:].rearrange("(c p) s -> p c s", p=P)

    with tile.TileContext(nc) as tc:
        with (
            tc.tile_pool(name="persist", bufs=1) as persist,
            tc.tile_pool(name="mm_ps", bufs=4, space="PSUM") as mm_ps,
            tc.tile_pool(name="tr_ps", bufs=4, space="PSUM") as tr_ps,
        ):
            # ---- persistent tiles -------------------------------------
            KT_sb = persist.tile([P, DC, S], bf16)          # K^T  [d | t]
            V_sb = persist.tile([P, TC, D], bf16)           # V natural [t | dv]
            QT_sb = persist.tile([P, DC, SH], bf16, tag="qt_ot")   # Q^T [d | s]
            ident = persist.tile([P, P], bf16)
            make_identity(nc, ident)
            # fp32r matmul operands must come from an instruction that
            # rounds to fp32r -> stage the DMA'd bias pack
            # (bq|bk|bv|bo|ones|rowsums) through a DVE copy.
            BALL_ld = persist.tile([1, BSZ], f32)
            nc.sync.dma_start(BALL_ld[:], BALL_d[:])
            BALL_r = persist.tile([1, BSZ], f32r)
            nc.vector.tensor_copy(out=BALL_r[:], in_=BALL_ld[:])
            BQ_sb = BALL_r[0:1, 0 * D:1 * D]
            BK_sb = BALL_r[0:1, 1 * D:2 * D]
            BV_sb = BALL_r[0:1, 2 * D:3 * D]
            BO_sb = BALL_r[0:1, 3 * D:4 * D]
            ONES = BALL_r[0:1, 4 * D:4 * D + NT]
            RS_sb = BALL_r[0:1, 4 * D + NT:BSZ]             # 1 + rowsum(I)

            # ---- phase A: QKV ----------------------------------------
            # XT shares its slot with attn^T (written only after XT's last
            # read); Wq/Wk/Wv double-buffer in a phase-scoped pool.
            XT_sb = persist.tile([P, DC, S], bf16, tag="xt_at")
            nc.sync.dma_start(XT_sb[:], xt_v)
            with tc.tile_pool(name="wpool", bufs=2) as wpool:
                WQ_sb = wpool.tile([P, DC, D], bf16, tag="w")
                nc.sync.dma_start(WQ_sb[:], wq_v)
                WK_sb = wpool.tile([P, DC, D], bf16, tag="w")
                nc.sync.dma_start(WK_sb[:], wk_v)

                # Q^T [dout, s-own]
                for c in range(DC):
                    for j in range(SJ):
                        ps = mm_ps.tile([P, NT], f32, tag="mm")
                        for dc in range(DC):
                            nc.tensor.matmul(
                                ps[:],
                                WQ_sb[:, dc, c * P:(c + 1) * P],
                                XT_sb[:, dc, j * NT:(j + 1) * NT],
                                start=(dc == 0), stop=False,
                            )
                        nc.tensor.matmul(
                            ps[:], BQ_sb[0:1, c * P:(c + 1) * P],
                            ONES[:], start=False, stop=True)
                        nc.vector.tensor_copy(
                            out=QT_sb[:, c, j * NT:(j + 1) * NT], in_=ps[:])
                # K^T [dout, t-full]
                for c in range(DC):
                    for j in range(TJ):
                        ps = mm_ps.tile([P, NT], f32, tag="mm")
                        for dc in range(DC):
                            nc.tensor.matmul(
                                ps[:],
                                WK_sb[:, dc, c * P:(c + 1) * P],
                                XT_sb[:, dc, j * NT:(j + 1) * NT],
                                start=(dc == 0), stop=False,
                            )
                        nc.tensor.matmul(
                            ps[:], BK_sb[0:1, c * P:(c + 1) * P],
                            ONES[:], start=False, stop=True)
                        nc.vector.tensor_copy(
                            out=KT_sb[:, c, j * NT:(j + 1) * NT], in_=ps[:])
                WV_sb = wpool.tile([P, DC, D], bf16, tag="w")
                nc.sync.dma_start(WV_sb[:], wv_v)
                # V natural [t, dv] (no bias; rank-1 correction at PV)
                for t in range(TC):
                    for j in range(D // NT):
                        ps = mm_ps.tile([P, NT], f32, tag="mm")
                        for dc in range(DC):
                            nc.tensor.matmul(
                                ps[:],
                                XT_sb[:, dc, t * P:(t + 1) * P],
                                WV_sb[:, dc, j * NT:(j + 1) * NT],
                                start=(dc == 0),
                                stop=(dc == DC - 1),
                            )
                        nc.vector.tensor_copy(
                            out=V_sb[:, t, j * NT:(j + 1) * NT], in_=ps[:])

            # ---- phase B/C: scores -> softmax -> +I^T -> attn^T -------
            AT_sb = persist.tile([P, TC, SH], bf16, tag="xt_at")  # attn^T
            with (
                tc.tile_pool(name="e_pool", bufs=2) as e_pool,
                tc.tile_pool(name="it_pool", bufs=3) as it_pool,
                tc.tile_pool(name="stat_pool", bufs=2) as stat_pool,
            ):
                for si in range(DC):  # 8 chunks of 128 own query rows
                    E_sb = e_pool.tile([P, S], bf16, tag="e")
                    acc4 = stat_pool.tile([P, TJ], f32, tag="acc")
                    IT_sb = it_pool.tile([P, TC, P], bf16, tag="it")
                    nc.sync.dma_start(
                        IT_sb[:], it_v[:, :, si * P:(si + 1) * P])
                    for tj in range(TJ):
                        ps = mm_ps.tile([P, NT], f32, tag="mm")
                        for dc in range(DC):
                            nc.tensor.matmul(
                                ps[:],
                                QT_sb[:, dc, si * P:(si + 1) * P],
                                KT_sb[:, dc, tj * NT:(tj + 1) * NT],
                                start=(dc == 0),
                                stop=(dc == DC - 1),
                            )
                        nc.scalar.activation(
                            E_sb[:, tj * NT:(tj + 1) * NT], ps[:], Exp,
                            scale=SCALE, accum_out=acc4[:, tj:tj + 1],
                        )
                    den = stat_pool.tile([P, 1], f32, tag="den")
                    recip = stat_pool.tile([P, 1], f32, tag="recip")
                    diag = stat_pool.tile([P, P], bf16, tag="diag")
                    nc.vector.reduce_sum(den[:], acc4[:], axis=mybir.AxisListType.X)
                    nc.vector.reciprocal(recip[:], den[:])
                    # diag(recip): identity rows scaled per-partition
                    nc.vector.tensor_scalar_mul(diag[:], ident[:], recip[:])
                    # attn^T tile = E-slice.T @ diag  (transpose + normalize),
                    # then the PSUM drain adds intensity^T on DVE.
                    for t in range(TC):
                        pt = tr_ps.tile([P, P], f32, tag="tr")
                        nc.tensor.matmul(
                            pt[:], E_sb[:, t * P:(t + 1) * P], diag[:],
                            start=True, stop=True)
                        nc.vector.tensor_tensor(
                            AT_sb[:, t, si * P:(si + 1) * P],
                            pt[:], IT_sb[:, t, :], add)

            # ---- phase D/E: PV -> out^T, then projection per s-tile ---
            OT_sb = persist.tile([P, DC, SH], bf16, tag="qt_ot")
            with (
                tc.tile_pool(name="wo_pool", bufs=1) as wo_pool,
                tc.tile_pool(name="fin_pool", bufs=3) as fin_pool,
            ):
                WO_sb = wo_pool.tile([P, DC, D], bf16)
                nc.sync.dma_start(WO_sb[:], wo_v)
                for sj in range(SJ):
                    for dvi in range(DC):
                        ps = mm_ps.tile([P, NT], f32, tag="mm")
                        for t in range(TC):
                            nc.tensor.matmul(
                                ps[:],
                                V_sb[:, t, dvi * P:(dvi + 1) * P],
                                AT_sb[:, t, sj * NT:(sj + 1) * NT],
                                start=(t == 0),
                                stop=False,
                            )
                        # bias: bv (x) (1 + rowsum(I))
                        nc.tensor.matmul(
                            ps[:], BV_sb[0:1, dvi * P:(dvi + 1) * P],
                            RS_sb[0:1, sj * NT:(sj + 1) * NT],
                            start=False, stop=True)
                        nc.vector.tensor_copy(
                            out=OT_sb[:, dvi, sj * NT:(sj + 1) * NT], in_=ps[:])

                    for doi in range(DC):
                        ps = mm_ps.tile([P, NT], f32, tag="mm")
                        for dvc in range(DC):
                            nc.tensor.matmul(
                                ps[:],
                                WO_sb[:, dvc, doi * P:(doi + 1) * P],
                                OT_sb[:, dvc, sj * NT:(sj + 1) * NT],
                                start=(dvc == 0), stop=False,
                            )
                        nc.tensor.matmul(
                            ps[:], BO_sb[0:1, doi * P:(doi + 1) * P],
                            ONES[:], start=False, stop=True)
                        F_sb = fin_pool.tile([P, NT], f32, tag="fin")
                        nc.vector.tensor_copy(out=F_sb[:], in_=ps[:])
                        nc.sync.dma_start(
                            out_v[:, doi, sj * NT:(sj + 1) * NT], F_sb[:])

    nc.compile()
    return nc


def _get_module():
    if "nc" not in _CACHE:
        _CACHE["nc"] = _build_module()
    return _CACHE["nc"]


def _make_in_maps(inputs):
    X = np.asarray(inputs["X"], dtype=np.float32)
    intensity = np.asarray(inputs["intensity"], dtype=np.float32)
    bf = ml_dtypes.bfloat16
    WqT = np.ascontiguousarray(np.asarray(inputs["Wq"], np.float32).T).astype(bf)
    WkT = np.ascontiguousarray(np.asarray(inputs["Wk"], np.float32).T).astype(bf)
    WvT = np.ascontiguousarray(np.asarray(inputs["Wv"], np.float32).T).astype(bf)
    WoT = np.ascontiguousarray(np.asarray(inputs["Wo"], np.float32).T).astype(bf)
    biases = [np.asarray(inputs[k], np.float32).reshape(D)
              for k in ("bq", "bk", "bv", "bo")]

    in_maps = []
    for c in range(8):
        b, h = c // 2, c % 2
        # Roll the t-axis so this core's own query rows sit at columns
        # 0..SH of XT; K/V/intensity follow the same rolled t-order, which
        # leaves attn @ V invariant.
        XT = np.ascontiguousarray(np.roll(X[b].T, -h * SH, axis=1)).astype(bf)
        Islc = np.roll(intensity[b, h * SH:(h + 1) * SH, :], -h * SH, axis=1)
        IT = np.ascontiguousarray(Islc.T).astype(bf)
        rows = 1.0 + Islc.sum(axis=1, dtype=np.float64).astype(np.float32)
        BALL = np.concatenate(
            biases + [np.ones(NT, np.float32), rows]).reshape(1, BSZ)
        in_maps.append({
            "XT": XT, "WQT": WqT, "WKT": WkT, "WVT": WvT, "WOT": WoT,
            "BALL": BALL, "IT": IT,
        })
    return in_maps


def _gather(results):
    out = np.empty((4, S, D), dtype=np.float32)
    for c in range(8):
        b, h = c // 2, c % 2
        out[b, h * SH:(h + 1) * SH, :] = results[c]["OUTT"].T
    return out


def kernel(**inputs):
    from concourse import bass_utils

    in_maps = _make_in_maps(inputs)
    nc = _get_module()
    res = bass_utils.run_bass_kernel_spmd(nc, in_maps, core_ids=list(range(8)))
    return _gather(res.results)


# revision 14
# speedup vs baseline: 1.1847x; 1.1217x over previous
"""Trainium2 Bass kernel for nn_Attention_54254026883778.

Single-head attention with an additive post-softmax intensity term:
    q/k/v = X @ W{q,k,v}.T + b;  scores = q k^T / sqrt(D)
    attn  = softmax(scores) + intensity;  out = (attn @ v) @ Wo.T + bo

Sharding: 8 cores = 4 batches x 2 sequence halves. Each core computes
K^T and V for its whole batch (duplicated across the pair) and Q/attention
for its own 1024 query rows. No collectives. The host rolls the t-axis per
core so the core's own query rows are always at t-positions 0..SH; K/V and
intensity follow the same rolled order, which leaves attn @ V invariant.

Device dataflow (host pre-transposes X, W and intensity so contraction /
partition dims land where the engines want them):
    Q^T/K^T  [dout | s]  = WxT-chunk.T @ XT        (lhsT=WxT, rhs=XT)
    V        [t | dv]    = XT-chunk.T @ WvT        (lhsT=XT,  rhs=WvT)
    scores   [s | t]     = QT-chunk.T @ KT  -> exp on ACT (no max-subtract:
        |scores| < ~3) with fused row-accumulate -> 1/den on DVE ->
        diag(recip) = ident * recip (DVE) ->
        attn^T tile = E-slice.T @ diag(recip)      (one PE matmul both
        transposes and normalizes) -> DVE copy adds intensity^T (bf16,
        host-transposed) while draining PSUM -> attn^T [t | s]
    out^T    [dv | s]    = V-chunk.T @ attn^T
    final^T  [do | s]    = WoT-chunk.T @ out^T     -> DRAM, host transposes
Biases enter as rank-1 fp32r matmuls accumulated into PSUM:
    q/k/o:  b_row (x) ones_row;   v: bv_row (x) attn-rowsums, where the
    rowsums are 1 + rowsum(intensity) (softmax rows sum to 1), shipped
    from the host inside the bias pack.
"""

import numpy as np
import ml_dtypes

P = 128
D = 1024
S = 2048          # keys per batch (full sequence)
SH = 1024         # query rows owned by each core
DC = D // P       # 8  contraction chunks over model dim
TC = S // P       # 16 t (key) chunks
NT = 512          # matmul moving free dim / psum bank
SJ = SH // NT     # 2  s-tiles of own rows
TJ = S // NT      # 4  t-tiles
SCALE = 1.0 / 32.0  # 1/sqrt(D)

_CACHE = {}


def _build_module():
    import concourse.bass as bass
    import concourse.tile as tile
    import concourse.mybir as mybir
    from concourse import bacc
    from concourse.masks import make_identity

    f32 = mybir.dt.float32
    f32r = mybir.dt.float32r
    bf16 = mybir.dt.bfloat16
    Exp = mybir.ActivationFunctionType.Exp
    add = mybir.AluOpType.add

    nc = bacc.Bacc("TRN2", target_bir_lowering=False, debug=False)

    XT_d = nc.dram_tensor("XT", [D, S], bf16, kind="ExternalInput")
    WQ_d = nc.dram_tensor("WQT", [D, D], bf16, kind="ExternalInput")
    WK_d = nc.dram_tensor("WKT", [D, D], bf16, kind="ExternalInput")
    WV_d = nc.dram_tensor("WVT", [D, D], bf16, kind="ExternalInput")
    WO_d = nc.dram_tensor("WOT", [D, D], bf16, kind="ExternalInput")
    BCOL_d = nc.dram_tensor("BCOL", [P, 3 * DC], f32, kind="ExternalInput")
    BROW_d = nc.dram_tensor("BROW", [1, D + SH], f32, kind="ExternalInput")
    IT_d = nc.dram_tensor("IT", [S, SH], bf16, kind="ExternalInput")
    OUT_d = nc.dram_tensor("OUTT", [D, SH], f32, kind="ExternalOutput")

    xt_v = XT_d[:].rearrange("(c p) s -> p c s", p=P)
    wq_v = WQ_d[:].rearrange("(c p) o -> p c o", p=P)
    wk_v = WK_d[:].rearrange("(c p) o -> p c o", p=P)
    wv_v = WV_d[:].rearrange("(c p) o -> p c o", p=P)
    wo_v = WO_d[:].rearrange("(c p) o -> p c o", p=P)
    it_v = IT_d[:].rearrange("(c p) s -> p c s", p=P)   # [t-part, tc, s]
    out_v = OUT_d[:].rearrange("(c p) s -> p c s", p=P)

    with tile.TileContext(nc) as tc:
        with (
            tc.tile_pool(name="persist", bufs=1) as persist,
            tc.tile_pool(name="mm_ps", bufs=4, space="PSUM") as mm_ps,
            tc.tile_pool(name="tr_ps", bufs=4, space="PSUM") as tr_ps,
        ):
            # ---- persistent tiles -------------------------------------
            KT_sb = persist.tile([P, DC, S], bf16)          # K^T  [d | t]
            V_sb = persist.tile([P, TC, D], bf16)           # V natural [t | dv]
            QT_sb = persist.tile([P, DC, SH], bf16, tag="qt_ot")   # Q^T [d | s]
            ident = persist.tile([P, P], bf16)
            make_identity(nc, ident)
            # bq|bk|bo as per-partition columns, added during PSUM extract
            BCOL_sb = persist.tile([P, 3 * DC], f32)
            nc.sync.dma_start(BCOL_sb[:], BCOL_d[:])
            # bv and the attn rowsums feed the PV rank-1 bias matmul; fp32r
            # operands must come from a rounding instruction, so stage the
            # DMA through a DVE copy.
            BROW_ld = persist.tile([1, D + SH], f32)
            nc.sync.dma_start(BROW_ld[:], BROW_d[:])
            BROW_r = persist.tile([1, D + SH], f32r)
            nc.vector.tensor_copy(out=BROW_r[:], in_=BROW_ld[:])
            BV_sb = BROW_r[0:1, 0:D]
            RS_sb = BROW_r[0:1, D:D + SH]                   # 1 + rowsum(I)

            # ---- phase A: QKV ----------------------------------------
            # XT shares its slot with attn^T (written only after XT's last
            # read); Wq/Wk/Wv double-buffer in a phase-scoped pool.
            XT_sb = persist.tile([P, DC, S], bf16, tag="xt_at")
            with tc.tile_pool(name="wpool", bufs=2) as wpool:
                WQ_sb = wpool.tile([P, DC, D], bf16, tag="w")
                WK_sb = wpool.tile([P, DC, D], bf16, tag="w")
                # Per-chunk DMAs so the first accumulation matmul only waits
                # for chunk 0 of XT/WQ (~0.75 MB) instead of the full 6 MB.
                for dc in range(DC):
                    nc.sync.dma_start(XT_sb[:, dc, :], xt_v[:, dc, :])
                    nc.sync.dma_start(WQ_sb[:, dc, :], wq_v[:, dc, :])
                for dc in range(DC):
                    nc.sync.dma_start(WK_sb[:, dc, :], wk_v[:, dc, :])

                # Q^T [dout, s-own]; dc outer so one stationary operand
                # serves SJ matmuls
                for c in range(DC):
                    psl = [mm_ps.tile([P, NT], f32, tag="mm", name="ps") for _ in range(SJ)]
                    for dc in range(DC):
                        for j in range(SJ):
                            nc.tensor.matmul(
                                psl[j][:],
                                WQ_sb[:, dc, c * P:(c + 1) * P],
                                XT_sb[:, dc, j * NT:(j + 1) * NT],
                                start=(dc == 0), stop=(dc == DC - 1),
                            )
                    for j in range(SJ):
                        nc.vector.tensor_scalar_add(
                            QT_sb[:, c, j * NT:(j + 1) * NT], psl[j][:],
                            BCOL_sb[:, c:c + 1])
                # K^T [dout, t-full]; one stationary serves TJ matmuls
                for c in range(DC):
                    psl = [mm_ps.tile([P, NT], f32, tag="mm", name="ps") for _ in range(TJ)]
                    for dc in range(DC):
                        for j in range(TJ):
                            nc.tensor.matmul(
                                psl[j][:],
                                WK_sb[:, dc, c * P:(c + 1) * P],
                                XT_sb[:, dc, j * NT:(j + 1) * NT],
                                start=(dc == 0), stop=(dc == DC - 1),
                            )
                    for j in range(TJ):
                        nc.vector.tensor_scalar_add(
                            KT_sb[:, c, j * NT:(j + 1) * NT], psl[j][:],
                            BCOL_sb[:, DC + c:DC + c + 1])
                WV_sb = wpool.tile([P, DC, D], bf16, tag="w")
                for dc in range(DC):
                    nc.sync.dma_start(WV_sb[:, dc, :], wv_v[:, dc, :])
                # V natural [t, dv] (no bias; rank-1 correction at PV);
                # one stationary serves both dv tiles
                for t in range(TC):
                    psl = [mm_ps.tile([P, NT], f32, tag="mm", name="ps")
                           for _ in range(D // NT)]
                    for dc in range(DC):
                        for j in range(D // NT):
                            nc.tensor.matmul(
                                psl[j][:],
                                XT_sb[:, dc, t * P:(t + 1) * P],
                                WV_sb[:, dc, j * NT:(j + 1) * NT],
                                start=(dc == 0),
                                stop=(dc == DC - 1),
                            )
                    for j in range(D // NT):
                        nc.vector.tensor_copy(
                            out=V_sb[:, t, j * NT:(j + 1) * NT], in_=psl[j][:])

            # ---- phase B/C: scores -> softmax -> +I^T -> attn^T -------
            AT_sb = persist.tile([P, TC, SH], bf16, tag="xt_at")  # attn^T
            with (
                tc.tile_pool(name="e_pool", bufs=2) as e_pool,
                tc.tile_pool(name="it_pool", bufs=3) as it_pool,
                tc.tile_pool(name="stat_pool", bufs=2) as stat_pool,
            ):
                for si in range(DC):  # 8 chunks of 128 own query rows
                    E_sb = e_pool.tile([P, S], bf16, tag="e")
                    acc4 = stat_pool.tile([P, TJ], f32, tag="acc")
                    IT_sb = it_pool.tile([P, TC, P], bf16, tag="it")
                    nc.sync.dma_start(
                        IT_sb[:], it_v[:, :, si * P:(si + 1) * P])
                    psl = [mm_ps.tile([P, NT], f32, tag="mm", name="ps")
                           for _ in range(TJ)]
                    for dc in range(DC):
                        for tj in range(TJ):
                            nc.tensor.matmul(
                                psl[tj][:],
                                QT_sb[:, dc, si * P:(si + 1) * P],
                                KT_sb[:, dc, tj * NT:(tj + 1) * NT],
                                start=(dc == 0),
                                stop=(dc == DC - 1),
                            )
                    for tj in range(TJ):
                        nc.scalar.activation(
                            E_sb[:, tj * NT:(tj + 1) * NT], psl[tj][:], Exp,
                            scale=SCALE, accum_out=acc4[:, tj:tj + 1],
                        )
                    den = stat_pool.tile([P, 1], f32, tag="den")
                    recip = stat_pool.tile([P, 1], f32, tag="recip")
                    diag = stat_pool.tile([P, P], bf16, tag="diag")
                    nc.vector.reduce_sum(den[:], acc4[:], axis=mybir.AxisListType.X)
                    nc.vector.reciprocal(recip[:], den[:])
                    # diag(recip): identity rows scaled per-partition
                    nc.vector.tensor_scalar_mul(diag[:], ident[:], recip[:])
                    # attn^T tile = E-slice.T @ diag  (transpose + normalize),
                    # then the PSUM drain adds intensity^T on DVE.
                    for t in range(TC):
                        pt = tr_ps.tile([P, P], f32, tag="tr")
                        nc.tensor.matmul(
                            pt[:], E_sb[:, t * P:(t + 1) * P], diag[:],
                            start=True, stop=True)
                        nc.vector.tensor_tensor(
                            AT_sb[:, t, si * P:(si + 1) * P],
                            pt[:], IT_sb[:, t, :], add)

            # ---- phase D/E: PV -> out^T, then projection per s-tile ---
            OT_sb = persist.tile([P, DC, SH], bf16, tag="qt_ot")
            with (
                tc.tile_pool(name="wo_pool", bufs=1) as wo_pool,
                tc.tile_pool(name="fin_pool", bufs=3) as fin_pool,
            ):
                WO_sb = wo_pool.tile([P, DC, D], bf16)
                nc.sync.dma_start(WO_sb[:], wo_v)
                for sj in range(SJ):
                    for dvi in range(DC):
                        ps = mm_ps.tile([P, NT], f32, tag="mm")
                        for t in range(TC):
                            nc.tensor.matmul(
                                ps[:],
                                V_sb[:, t, dvi * P:(dvi + 1) * P],
                                AT_sb[:, t, sj * NT:(sj + 1) * NT],
                                start=(t == 0),
                                stop=False,
                            )
                        # bias: bv (x) (1 + rowsum(I))
                        nc.tensor.matmul(
                            ps[:], BV_sb[0:1, dvi * P:(dvi + 1) * P],
                            RS_sb[0:1, sj * NT:(sj + 1) * NT],
                            start=False, stop=True)
                        nc.vector.tensor_copy(
                            out=OT_sb[:, dvi, sj * NT:(sj + 1) * NT], in_=ps[:])

                    for doi in range(DC):
                        ps = mm_ps.tile([P, NT], f32, tag="mm")
                        for dvc in range(DC):
                            nc.tensor.matmul(
                                ps[:],
                                WO_sb[:, dvc, doi * P:(doi + 1) * P],
                                OT_sb[:, dvc, sj * NT:(sj + 1) * NT],
                                start=(dvc == 0), stop=(dvc == DC - 1),
                            )
                        F_sb = fin_pool.tile([P, NT], f32, tag="fin")
                        nc.vector.tensor_scalar_add(
                            F_sb[:], ps[:],
                            BCOL_sb[:, 2 * DC + doi:2 * DC + doi + 1])
                        nc.sync.dma_start(
                            out_v[:, doi, sj * NT:(sj + 1) * NT], F_sb[:])

    nc.compile()
    return nc


def _get_module():
    if "nc" not in _CACHE:
        _CACHE["nc"] = _build_module()
    return _CACHE["nc"]


def _make_in_maps(inputs):
    X = np.asarray(inputs["X"], dtype=np.float32)
    intensity = np.asarray(inputs["intensity"], dtype=np.float32)
    bf = ml_dtypes.bfloat16
    WqT = np.ascontiguousarray(np.asarray(inputs["Wq"], np.float32).T).astype(bf)
    WkT = np.ascontiguousarray(np.asarray(inputs["Wk"], np.float32).T).astype(bf)
    WvT = np.ascontiguousarray(np.asarray(inputs["Wv"], np.float32).T).astype(bf)
    WoT = np.ascontiguousarray(np.asarray(inputs["Wo"], np.float32).T).astype(bf)
    bq, bk, bv, bo = (np.asarray(inputs[k], np.float32).reshape(D)
                      for k in ("bq", "bk", "bv", "bo"))
    BCOL = np.concatenate(
        [b.reshape(DC, P).T for b in (bq, bk, bo)], axis=1
    ).astype(np.float32)  # [128, 24]

    in_maps = []
    for c in range(8):
        b, h = c // 2, c % 2
        # Roll the t-axis so this core's own query rows sit at columns
        # 0..SH of XT; K/V/intensity follow the same rolled t-order, which
        # leaves attn @ V invariant.
        XT = np.ascontiguousarray(np.roll(X[b].T, -h * SH, axis=1)).astype(bf)
        Islc = np.roll(intensity[b, h * SH:(h + 1) * SH, :], -h * SH, axis=1)
        IT = np.ascontiguousarray(Islc.T).astype(bf)
        rows = 1.0 + Islc.sum(axis=1, dtype=np.float64).astype(np.float32)
        BROW = np.concatenate([bv, rows]).reshape(1, D + SH)
        in_maps.append({
            "XT": XT, "WQT": WqT, "WKT": WkT, "WVT": WvT, "WOT": WoT,
            "BCOL": BCOL, "BROW": BROW, "IT": IT,
        })
    return in_maps


def _gather(results):
    out = np.empty((4, S, D), dtype=np.float32)
    for c in range(8):
        b, h = c // 2, c % 2
        out[b, h * SH:(h + 1) * SH, :] = results[c]["OUTT"].T
    return out


def kernel(**inputs):
    from concourse import bass_utils

    in_maps = _make_in_maps(inputs)
    nc = _get_module()
    res = bass_utils.run_bass_kernel_spmd(nc, in_maps, core_ids=list(range(8)))
    return _gather(res.results)
